# revision 2
# baseline (speedup 1.0000x reference)
"""Trainium2 Bass kernel for a GNN MetaLayer (edge MLP -> segment-sum -> node MLP).

Distribution: edge-parallel across 8 NeuronCores (contiguous 62500-edge shards,
x + weights replicated), node MLP node-parallel (contiguous 6250-node shards).
Device does all matmul work feature-major in bf16 with fp32 PSUM accumulation;
host prepares feature-major operands and performs the (memory-bound) segment-sum
between the two launches via sorted reduceat.
"""
import sys, os
sys.path.insert(0, "/opt/trn_rl_repo")
import numpy as np
import ml_dtypes
from contextlib import ExitStack

import concourse.bass as bass
import concourse.bacc as bacc
import concourse.mybir as mybir
import concourse.tile as tile
from concourse.bass_utils import run_bass_kernel_spmd

BF16 = ml_dtypes.bfloat16

N_NODES = 50000
N_EDGES = 500000
D = 128
H = 256
NCORES = 8
E_SH = N_EDGES // NCORES            # 62500
UNIT = 512                           # edges per inner unit (PSUM free dim)
E_PAD = ((E_SH + UNIT - 1) // UNIT) * UNIT   # 62976
CHUNK = 2560                         # edges per DMA chunk
N_SH = N_NODES // NCORES             # 6250
N_PAD = ((N_SH + UNIT - 1) // UNIT) * UNIT   # 6656

_cache = {}


def _build_edge_program():
    nc = bacc.Bacc(None)
    rowT = nc.dram_tensor("rowT", [128, E_PAD], mybir.dt.bfloat16, kind="ExternalInput")
    colT = nc.dram_tensor("colT", [128, E_PAD], mybir.dt.bfloat16, kind="ExternalInput")
    attrT = nc.dram_tensor("attrT", [128, E_PAD], mybir.dt.bfloat16, kind="ExternalInput")
    w1 = nc.dram_tensor("w1", [128, 3 * H], mybir.dt.bfloat16, kind="ExternalInput")
    w2 = nc.dram_tensor("w2", [128, 2 * D], mybir.dt.bfloat16, kind="ExternalInput")
    bia = nc.dram_tensor("bia", [128, 3], mybir.dt.float32, kind="ExternalInput")
    enT = nc.dram_tensor("enT", [128, E_PAD], mybir.dt.bfloat16, kind="ExternalOutput")

    n_chunks = E_PAD // CHUNK if E_PAD % CHUNK == 0 else None
    # E_PAD=62976 is not a multiple of 2560; use ragged chunk list of UNIT multiples
    chunks = []
    off = 0
    while off < E_PAD:
        sz = min(CHUNK, E_PAD - off)
        chunks.append((off, sz))
        off += sz

    with ExitStack() as ctx:
        tc = ctx.enter_context(tile.TileContext(nc))
        wpool = ctx.enter_context(tc.tile_pool(name="w", bufs=1))
        io = ctx.enter_context(tc.tile_pool(name="io", bufs=2))
        mid = ctx.enter_context(tc.tile_pool(name="mid", bufs=3))
        ps1 = ctx.enter_context(tc.tile_pool(name="ps1", bufs=2, space="PSUM"))
        ps2 = ctx.enter_context(tc.tile_pool(name="ps2", bufs=2, space="PSUM"))

        t_w1 = wpool.tile([128, 3 * H], mybir.dt.bfloat16)
        nc.sync.dma_start(out=t_w1[:], in_=w1[:])
        t_w2 = wpool.tile([128, 2 * D], mybir.dt.bfloat16)
        nc.sync.dma_start(out=t_w2[:], in_=w2[:])
        t_b = wpool.tile([128, 3], mybir.dt.float32)
        nc.sync.dma_start(out=t_b[:], in_=bia[:])

        for (coff, csz) in chunks:
            t_row = io.tile([128, CHUNK], mybir.dt.bfloat16, tag="row")
            t_col = io.tile([128, CHUNK], mybir.dt.bfloat16, tag="col")
            t_attr = io.tile([128, CHUNK], mybir.dt.bfloat16, tag="attr")
            nc.sync.dma_start(out=t_row[:, :csz], in_=rowT[:, coff:coff + csz])
            nc.sync.dma_start(out=t_col[:, :csz], in_=colT[:, coff:coff + csz])
            nc.sync.dma_start(out=t_attr[:, :csz], in_=attrT[:, coff:coff + csz])
            t_eno = mid.tile([128, CHUNK], mybir.dt.bfloat16, tag="eno")
            for u in range(csz // UNIT):
                s = slice(u * UNIT, (u + 1) * UNIT)
                # --- mm1: e_h[Mh] = relu(sum_k W1[k,Mh]^T @ in_k + b1[Mh]) ---
                t_eh = mid.tile([128, 2 * UNIT], mybir.dt.bfloat16, tag="eh")
                for mh in range(2):
                    p1 = ps1.tile([128, UNIT], mybir.dt.float32, tag="p1")
                    for k, t_in in enumerate((t_row, t_col, t_attr)):
                        nc.tensor.matmul(
                            out=p1[:],
                            lhsT=t_w1[:, k * H + mh * 128: k * H + mh * 128 + 128],
                            rhs=t_in[:, s],
                            start=(k == 0),
                            stop=(k == 2),
                        )
                    nc.scalar.activation(
                        out=t_eh[:, mh * UNIT:(mh + 1) * UNIT],
                        in_=p1[:],
                        func=mybir.ActivationFunctionType.Relu,
                        bias=t_b[:, mh:mh + 1],
                    )
                # --- mm2: e_new^T = W2^T @ e_h + b2 ---
                p2 = ps2.tile([128, UNIT], mybir.dt.float32, tag="p2")
                for k in range(2):
                    nc.tensor.matmul(
                        out=p2[:],
                        lhsT=t_w2[:, k * D:(k + 1) * D],
                        rhs=t_eh[:, k * UNIT:(k + 1) * UNIT],
                        start=(k == 0),
                        stop=(k == 1),
                    )
                nc.vector.tensor_scalar_add(out=t_eno[:, u * UNIT:(u + 1) * UNIT], in0=p2[:], scalar1=t_b[:, 2:3])
            nc.sync.dma_start(out=enT[:, coff:coff + csz], in_=t_eno[:, :csz])
    nc.compile()
    return nc


def _build_node_program():
    nc = bacc.Bacc(None)
    xT = nc.dram_tensor("xT", [128, N_PAD], mybir.dt.bfloat16, kind="ExternalInput")
    aggT = nc.dram_tensor("aggT", [128, N_PAD], mybir.dt.bfloat16, kind="ExternalInput")
    w1 = nc.dram_tensor("w1", [128, 2 * H], mybir.dt.bfloat16, kind="ExternalInput")
    w2 = nc.dram_tensor("w2", [128, 2 * D], mybir.dt.bfloat16, kind="ExternalInput")
    bia = nc.dram_tensor("bia", [128, 3], mybir.dt.float32, kind="ExternalInput")
    xnT = nc.dram_tensor("xnT", [128, N_PAD], mybir.dt.float32, kind="ExternalOutput")

    with ExitStack() as ctx:
        tc = ctx.enter_context(tile.TileContext(nc))
        wpool = ctx.enter_context(tc.tile_pool(name="w", bufs=1))
        io = ctx.enter_context(tc.tile_pool(name="io", bufs=2))
        mid = ctx.enter_context(tc.tile_pool(name="mid", bufs=3))
        ps1 = ctx.enter_context(tc.tile_pool(name="ps1", bufs=2, space="PSUM"))
        ps2 = ctx.enter_context(tc.tile_pool(name="ps2", bufs=2, space="PSUM"))

        t_w1 = wpool.tile([128, 2 * H], mybir.dt.bfloat16)
        nc.sync.dma_start(out=t_w1[:], in_=w1[:])
        t_w2 = wpool.tile([128, 2 * D], mybir.dt.bfloat16)
        nc.sync.dma_start(out=t_w2[:], in_=w2[:])
        t_b = wpool.tile([128, 3], mybir.dt.float32)
        nc.sync.dma_start(out=t_b[:], in_=bia[:])

        off = 0
        while off < N_PAD:
            csz = min(CHUNK, N_PAD - off)
            t_x = io.tile([128, CHUNK], mybir.dt.bfloat16, tag="x")
            t_a = io.tile([128, CHUNK], mybir.dt.bfloat16, tag="a")
            nc.sync.dma_start(out=t_x[:, :csz], in_=xT[:, off:off + csz])
            nc.sync.dma_start(out=t_a[:, :csz], in_=aggT[:, off:off + csz])
            for u in range(csz // UNIT):
                s = slice(u * UNIT, (u + 1) * UNIT)
                t_nh = mid.tile([128, 2 * UNIT], mybir.dt.bfloat16, tag="nh")
                for mh in range(2):
                    p1 = ps1.tile([128, UNIT], mybir.dt.float32, tag="p1")
                    for k, t_in in enumerate((t_x, t_a)):
                        nc.tensor.matmul(
                            out=p1[:],
                            lhsT=t_w1[:, k * H + mh * 128: k * H + mh * 128 + 128],
                            rhs=t_in[:, s],
                            start=(k == 0),
                            stop=(k == 1),
                        )
                    nc.scalar.activation(
                        out=t_nh[:, mh * UNIT:(mh + 1) * UNIT],
                        in_=p1[:],
                        func=mybir.ActivationFunctionType.Relu,
                        bias=t_b[:, mh:mh + 1],
                    )
                p2 = ps2.tile([128, UNIT], mybir.dt.float32, tag="p2")
                for k in range(2):
                    nc.tensor.matmul(
                        out=p2[:],
                        lhsT=t_w2[:, k * D:(k + 1) * D],
                        rhs=t_nh[:, k * UNIT:(k + 1) * UNIT],
                        start=(k == 0),
                        stop=(k == 1),
                    )
                t_xn = mid.tile([128, UNIT], mybir.dt.float32, tag="xn")
                nc.vector.tensor_scalar_add(out=t_xn[:], in0=p2[:], scalar1=t_b[:, 2:3])
                nc.sync.dma_start(out=xnT[:, off + u * UNIT: off + (u + 1) * UNIT], in_=t_xn[:])
            off += csz
    nc.compile()
    return nc


def _get_programs():
    if "edge" not in _cache:
        _cache["edge"] = _build_edge_program()
        _cache["node"] = _build_node_program()
    return _cache["edge"], _cache["node"]


def _pad_cols(a: np.ndarray, width: int) -> np.ndarray:
    out = np.zeros((a.shape[0], width), dtype=a.dtype)
    out[:, : a.shape[1]] = a
    return out


def kernel(x, edge_index, edge_attr, eW1, eb1, eW2, eb2, nW1, nb1, nW2, nb2,
           trace=False, _timing=None):
    x = np.asarray(x); edge_index = np.asarray(edge_index); edge_attr = np.asarray(edge_attr)
    eW1 = np.asarray(eW1); eb1 = np.asarray(eb1); eW2 = np.asarray(eW2); eb2 = np.asarray(eb2)
    nW1 = np.asarray(nW1); nb1 = np.asarray(nb1); nW2 = np.asarray(nW2); nb2 = np.asarray(nb2)
    row, col = edge_index[0].astype(np.int64), edge_index[1].astype(np.int64)

    nc_edge, nc_node = _get_programs()

    # ---- host prep (sharding): feature-major bf16 operands ----
    xT_bf = np.ascontiguousarray(x.astype(BF16).T)               # [128, N]
    attr_bf = edge_attr.astype(BF16)                              # [E, 128]

    # edge-program weights: w1 [128, 3*H] = eW1 K-chunks; w2 [128, 2*D]
    w1_e = np.ascontiguousarray(eW1.astype(BF16).reshape(3, 128, H).transpose(1, 0, 2)).reshape(128, 3 * H)
    w2_e = np.ascontiguousarray(eW2.astype(BF16).reshape(2, 128, D).transpose(1, 0, 2)).reshape(128, 2 * D)
    bia_e = np.stack([eb1[:128], eb1[128:], eb2], axis=1).astype(np.float32)  # [128, 3]

    in_maps = []
    for c in range(NCORES):
        e0, e1 = c * E_SH, (c + 1) * E_SH
        rc, cc = row[e0:e1], col[e0:e1]
        in_maps.append(dict(
            rowT=_pad_cols(xT_bf[:, rc], E_PAD),
            colT=_pad_cols(xT_bf[:, cc], E_PAD),
            attrT=_pad_cols(np.ascontiguousarray(attr_bf[e0:e1].T), E_PAD),
            w1=w1_e, w2=w2_e, bia=bia_e,
        ))

    res1 = run_bass_kernel_spmd(nc_edge, in_maps, core_ids=list(range(NCORES)),
                                trace=trace)
    if _timing is not None:
        _timing.append(res1)

    # ---- assemble e_new + host segment-sum ----
    e_new = np.empty((N_EDGES, D), dtype=np.float32)
    for c in range(NCORES):
        e_new[c * E_SH:(c + 1) * E_SH] = res1.results[c]["enT"][:, :E_SH].T.astype(np.float32)

    perm = np.argsort(col, kind="stable")
    sc = col[perm]
    starts = np.flatnonzero(np.r_[True, sc[1:] != sc[:-1]])
    sums = np.add.reduceat(e_new[perm], starts, axis=0)
    agg = np.zeros((N_NODES, D), dtype=np.float32)
    agg[sc[starts]] = sums

    # ---- node program ----
    aggT_bf = np.ascontiguousarray(agg.astype(BF16).T)            # [128, N]
    w1_n = np.ascontiguousarray(nW1.astype(BF16).reshape(2, 128, H).transpose(1, 0, 2)).reshape(128, 2 * H)
    w2_n = np.ascontiguousarray(nW2.astype(BF16).reshape(2, 128, D).transpose(1, 0, 2)).reshape(128, 2 * D)
    bia_n = np.stack([nb1[:128], nb1[128:], nb2], axis=1).astype(np.float32)

    in_maps2 = []
    for c in range(NCORES):
        n0, n1 = c * N_SH, (c + 1) * N_SH
        in_maps2.append(dict(
            xT=_pad_cols(xT_bf[:, n0:n1], N_PAD),
            aggT=_pad_cols(aggT_bf[:, n0:n1], N_PAD),
            w1=w1_n, w2=w2_n, bia=bia_n,
        ))
    res2 = run_bass_kernel_spmd(nc_node, in_maps2, core_ids=list(range(NCORES)),
                                trace=trace)
    if _timing is not None:
        _timing.append(res2)

    x_new = np.empty((N_NODES, D), dtype=np.float32)
    for c in range(NCORES):
        x_new[c * N_SH:(c + 1) * N_SH] = res2.results[c]["xnT"][:, :N_SH].T

    return (x_new, e_new)


# revision 3
# speedup vs baseline: 1.0057x; 1.0057x over previous
"""Trainium2 Bass kernel for a GNN MetaLayer (edge MLP -> segment-sum -> node MLP).

Distribution: edge-parallel across 8 NeuronCores (contiguous 62500-edge shards,
x + weights replicated), node MLP node-parallel (contiguous 6250-node shards).
Device does all matmul work feature-major in bf16 with fp32 PSUM accumulation;
host prepares feature-major operands and performs the (memory-bound) segment-sum
between the two launches via sorted reduceat.
"""
import sys, os
sys.path.insert(0, "/opt/trn_rl_repo")
import numpy as np
import ml_dtypes
from contextlib import ExitStack

import concourse.bass as bass
import concourse.bacc as bacc
import concourse.mybir as mybir
import concourse.tile as tile
from concourse.bass_utils import run_bass_kernel_spmd

BF16 = ml_dtypes.bfloat16

N_NODES = 50000
N_EDGES = 500000
D = 128
H = 256
NCORES = 8
E_SH = N_EDGES // NCORES            # 62500
UNIT = 512                           # edges per inner unit (PSUM free dim)
E_PAD = ((E_SH + UNIT - 1) // UNIT) * UNIT   # 62976
CHUNK = 2560                         # edges per DMA chunk
N_SH = N_NODES // NCORES             # 6250
N_PAD = ((N_SH + UNIT - 1) // UNIT) * UNIT   # 6656

_cache = {}


def _build_edge_program():
    nc = bacc.Bacc(None)
    rowT = nc.dram_tensor("rowT", [128, E_PAD], mybir.dt.bfloat16, kind="ExternalInput")
    colT = nc.dram_tensor("colT", [128, E_PAD], mybir.dt.bfloat16, kind="ExternalInput")
    attrT = nc.dram_tensor("attrT", [128, E_PAD], mybir.dt.bfloat16, kind="ExternalInput")
    w1 = nc.dram_tensor("w1", [128, 3 * H], mybir.dt.bfloat16, kind="ExternalInput")
    w2 = nc.dram_tensor("w2", [128, 2 * D], mybir.dt.bfloat16, kind="ExternalInput")
    bia = nc.dram_tensor("bia", [128, 3], mybir.dt.float32, kind="ExternalInput")
    enT = nc.dram_tensor("enT", [128, E_PAD], mybir.dt.bfloat16, kind="ExternalOutput")

    n_chunks = E_PAD // CHUNK if E_PAD % CHUNK == 0 else None
    # E_PAD=62976 is not a multiple of 2560; use ragged chunk list of UNIT multiples
    chunks = []
    off = 0
    while off < E_PAD:
        sz = min(CHUNK, E_PAD - off)
        chunks.append((off, sz))
        off += sz

    with ExitStack() as ctx:
        tc = ctx.enter_context(tile.TileContext(nc))
        wpool = ctx.enter_context(tc.tile_pool(name="w", bufs=1))
        io = ctx.enter_context(tc.tile_pool(name="io", bufs=2))
        mid = ctx.enter_context(tc.tile_pool(name="mid", bufs=3))
        ps1 = ctx.enter_context(tc.tile_pool(name="ps1", bufs=4, space="PSUM"))
        ps2 = ctx.enter_context(tc.tile_pool(name="ps2", bufs=4, space="PSUM"))

        t_w1 = wpool.tile([128, 3 * H], mybir.dt.bfloat16)
        nc.sync.dma_start(out=t_w1[:], in_=w1[:])
        t_w2 = wpool.tile([128, 2 * D], mybir.dt.bfloat16)
        nc.sync.dma_start(out=t_w2[:], in_=w2[:])
        t_b = wpool.tile([128, 3], mybir.dt.float32)
        nc.sync.dma_start(out=t_b[:], in_=bia[:])

        for (coff, csz) in chunks:
            t_row = io.tile([128, CHUNK], mybir.dt.bfloat16, tag="row")
            t_col = io.tile([128, CHUNK], mybir.dt.bfloat16, tag="col")
            t_attr = io.tile([128, CHUNK], mybir.dt.bfloat16, tag="attr")
            nc.sync.dma_start(out=t_row[:, :csz], in_=rowT[:, coff:coff + csz])
            nc.sync.dma_start(out=t_col[:, :csz], in_=colT[:, coff:coff + csz])
            nc.sync.dma_start(out=t_attr[:, :csz], in_=attrT[:, coff:coff + csz])
            t_eno = mid.tile([128, CHUNK], mybir.dt.bfloat16, tag="eno")
            for u in range(csz // UNIT):
                s = slice(u * UNIT, (u + 1) * UNIT)
                # --- mm1: e_h[Mh] = relu(sum_k W1[k,Mh]^T @ in_k + b1[Mh]) ---
                t_eh = mid.tile([128, 2 * UNIT], mybir.dt.bfloat16, tag="eh")
                for mh in range(2):
                    p1 = ps1.tile([128, UNIT], mybir.dt.float32, tag="p1")
                    for k, t_in in enumerate((t_row, t_col, t_attr)):
                        nc.tensor.matmul(
                            out=p1[:],
                            lhsT=t_w1[:, k * H + mh * 128: k * H + mh * 128 + 128],
                            rhs=t_in[:, s],
                            start=(k == 0),
                            stop=(k == 2),
                        )
                    nc.scalar.activation(
                        out=t_eh[:, mh * UNIT:(mh + 1) * UNIT],
                        in_=p1[:],
                        func=mybir.ActivationFunctionType.Relu,
                        bias=t_b[:, mh:mh + 1],
                    )
                # --- mm2: e_new^T = W2^T @ e_h + b2 ---
                p2 = ps2.tile([128, UNIT], mybir.dt.float32, tag="p2")
                for k in range(2):
                    nc.tensor.matmul(
                        out=p2[:],
                        lhsT=t_w2[:, k * D:(k + 1) * D],
                        rhs=t_eh[:, k * UNIT:(k + 1) * UNIT],
                        start=(k == 0),
                        stop=(k == 1),
                    )
                nc.vector.tensor_scalar_add(out=t_eno[:, u * UNIT:(u + 1) * UNIT], in0=p2[:], scalar1=t_b[:, 2:3])
            nc.sync.dma_start(out=enT[:, coff:coff + csz], in_=t_eno[:, :csz])
    nc.compile()
    return nc


def _build_node_program():
    nc = bacc.Bacc(None)
    xT = nc.dram_tensor("xT", [128, N_PAD], mybir.dt.bfloat16, kind="ExternalInput")
    aggT = nc.dram_tensor("aggT", [128, N_PAD], mybir.dt.bfloat16, kind="ExternalInput")
    w1 = nc.dram_tensor("w1", [128, 2 * H], mybir.dt.bfloat16, kind="ExternalInput")
    w2 = nc.dram_tensor("w2", [128, 2 * D], mybir.dt.bfloat16, kind="ExternalInput")
    bia = nc.dram_tensor("bia", [128, 3], mybir.dt.float32, kind="ExternalInput")
    xnT = nc.dram_tensor("xnT", [128, N_PAD], mybir.dt.float32, kind="ExternalOutput")

    with ExitStack() as ctx:
        tc = ctx.enter_context(tile.TileContext(nc))
        wpool = ctx.enter_context(tc.tile_pool(name="w", bufs=1))
        io = ctx.enter_context(tc.tile_pool(name="io", bufs=2))
        mid = ctx.enter_context(tc.tile_pool(name="mid", bufs=3))
        ps1 = ctx.enter_context(tc.tile_pool(name="ps1", bufs=2, space="PSUM"))
        ps2 = ctx.enter_context(tc.tile_pool(name="ps2", bufs=2, space="PSUM"))

        t_w1 = wpool.tile([128, 2 * H], mybir.dt.bfloat16)
        nc.sync.dma_start(out=t_w1[:], in_=w1[:])
        t_w2 = wpool.tile([128, 2 * D], mybir.dt.bfloat16)
        nc.sync.dma_start(out=t_w2[:], in_=w2[:])
        t_b = wpool.tile([128, 3], mybir.dt.float32)
        nc.sync.dma_start(out=t_b[:], in_=bia[:])

        off = 0
        while off < N_PAD:
            csz = min(CHUNK, N_PAD - off)
            t_x = io.tile([128, CHUNK], mybir.dt.bfloat16, tag="x")
            t_a = io.tile([128, CHUNK], mybir.dt.bfloat16, tag="a")
            nc.sync.dma_start(out=t_x[:, :csz], in_=xT[:, off:off + csz])
            nc.sync.dma_start(out=t_a[:, :csz], in_=aggT[:, off:off + csz])
            for u in range(csz // UNIT):
                s = slice(u * UNIT, (u + 1) * UNIT)
                t_nh = mid.tile([128, 2 * UNIT], mybir.dt.bfloat16, tag="nh")
                for mh in range(2):
                    p1 = ps1.tile([128, UNIT], mybir.dt.float32, tag="p1")
                    for k, t_in in enumerate((t_x, t_a)):
                        nc.tensor.matmul(
                            out=p1[:],
                            lhsT=t_w1[:, k * H + mh * 128: k * H + mh * 128 + 128],
                            rhs=t_in[:, s],
                            start=(k == 0),
                            stop=(k == 1),
                        )
                    nc.scalar.activation(
                        out=t_nh[:, mh * UNIT:(mh + 1) * UNIT],
                        in_=p1[:],
                        func=mybir.ActivationFunctionType.Relu,
                        bias=t_b[:, mh:mh + 1],
                    )
                p2 = ps2.tile([128, UNIT], mybir.dt.float32, tag="p2")
                for k in range(2):
                    nc.tensor.matmul(
                        out=p2[:],
                        lhsT=t_w2[:, k * D:(k + 1) * D],
                        rhs=t_nh[:, k * UNIT:(k + 1) * UNIT],
                        start=(k == 0),
                        stop=(k == 1),
                    )
                t_xn = mid.tile([128, UNIT], mybir.dt.float32, tag="xn")
                nc.vector.tensor_scalar_add(out=t_xn[:], in0=p2[:], scalar1=t_b[:, 2:3])
                nc.sync.dma_start(out=xnT[:, off + u * UNIT: off + (u + 1) * UNIT], in_=t_xn[:])
            off += csz
    nc.compile()
    return nc


def _get_programs():
    if "edge" not in _cache:
        _cache["edge"] = _build_edge_program()
        _cache["node"] = _build_node_program()
    return _cache["edge"], _cache["node"]


def _pad_cols(a: np.ndarray, width: int) -> np.ndarray:
    out = np.zeros((a.shape[0], width), dtype=a.dtype)
    out[:, : a.shape[1]] = a
    return out


def kernel(x, edge_index, edge_attr, eW1, eb1, eW2, eb2, nW1, nb1, nW2, nb2,
           trace=False, _timing=None):
    x = np.asarray(x); edge_index = np.asarray(edge_index); edge_attr = np.asarray(edge_attr)
    eW1 = np.asarray(eW1); eb1 = np.asarray(eb1); eW2 = np.asarray(eW2); eb2 = np.asarray(eb2)
    nW1 = np.asarray(nW1); nb1 = np.asarray(nb1); nW2 = np.asarray(nW2); nb2 = np.asarray(nb2)
    row, col = edge_index[0].astype(np.int64), edge_index[1].astype(np.int64)

    nc_edge, nc_node = _get_programs()

    # ---- host prep (sharding): feature-major bf16 operands ----
    xT_bf = np.ascontiguousarray(x.astype(BF16).T)               # [128, N]
    attr_bf = edge_attr.astype(BF16)                              # [E, 128]

    # edge-program weights: w1 [128, 3*H] = eW1 K-chunks; w2 [128, 2*D]
    w1_e = np.ascontiguousarray(eW1.astype(BF16).reshape(3, 128, H).transpose(1, 0, 2)).reshape(128, 3 * H)
    w2_e = np.ascontiguousarray(eW2.astype(BF16).reshape(2, 128, D).transpose(1, 0, 2)).reshape(128, 2 * D)
    bia_e = np.stack([eb1[:128], eb1[128:], eb2], axis=1).astype(np.float32)  # [128, 3]

    in_maps = []
    for c in range(NCORES):
        e0, e1 = c * E_SH, (c + 1) * E_SH
        rc, cc = row[e0:e1], col[e0:e1]
        in_maps.append(dict(
            rowT=_pad_cols(xT_bf[:, rc], E_PAD),
            colT=_pad_cols(xT_bf[:, cc], E_PAD),
            attrT=_pad_cols(np.ascontiguousarray(attr_bf[e0:e1].T), E_PAD),
            w1=w1_e, w2=w2_e, bia=bia_e,
        ))

    res1 = run_bass_kernel_spmd(nc_edge, in_maps, core_ids=list(range(NCORES)),
                                trace=trace)
    if _timing is not None:
        _timing.append(res1)

    # ---- assemble e_new + host segment-sum ----
    e_new = np.empty((N_EDGES, D), dtype=np.float32)
    for c in range(NCORES):
        e_new[c * E_SH:(c + 1) * E_SH] = res1.results[c]["enT"][:, :E_SH].T.astype(np.float32)

    perm = np.argsort(col, kind="stable")
    sc = col[perm]
    starts = np.flatnonzero(np.r_[True, sc[1:] != sc[:-1]])
    sums = np.add.reduceat(e_new[perm], starts, axis=0)
    agg = np.zeros((N_NODES, D), dtype=np.float32)
    agg[sc[starts]] = sums

    # ---- node program ----
    aggT_bf = np.ascontiguousarray(agg.astype(BF16).T)            # [128, N]
    w1_n = np.ascontiguousarray(nW1.astype(BF16).reshape(2, 128, H).transpose(1, 0, 2)).reshape(128, 2 * H)
    w2_n = np.ascontiguousarray(nW2.astype(BF16).reshape(2, 128, D).transpose(1, 0, 2)).reshape(128, 2 * D)
    bia_n = np.stack([nb1[:128], nb1[128:], nb2], axis=1).astype(np.float32)

    in_maps2 = []
    for c in range(NCORES):
        n0, n1 = c * N_SH, (c + 1) * N_SH
        in_maps2.append(dict(
            xT=_pad_cols(xT_bf[:, n0:n1], N_PAD),
            aggT=_pad_cols(aggT_bf[:, n0:n1], N_PAD),
            w1=w1_n, w2=w2_n, bia=bia_n,
        ))
    res2 = run_bass_kernel_spmd(nc_node, in_maps2, core_ids=list(range(NCORES)),
                                trace=trace)
    if _timing is not None:
        _timing.append(res2)

    x_new = np.empty((N_NODES, D), dtype=np.float32)
    for c in range(NCORES):
        x_new[c * N_SH:(c + 1) * N_SH] = res2.results[c]["xnT"][:, :N_SH].T

    return (x_new, e_new)


# revision 4
# speedup vs baseline: 1.0181x; 1.0123x over previous
"""Trainium2 Bass kernel for a GNN MetaLayer (edge MLP -> segment-sum -> node MLP).

Distribution: edge-parallel across 8 NeuronCores (contiguous 62500-edge shards,
x + weights replicated), node MLP node-parallel (contiguous 6250-node shards).
Device does all matmul work feature-major in bf16 with fp32 PSUM accumulation;
host prepares feature-major operands and performs the (memory-bound) segment-sum
between the two launches via sorted reduceat.
"""
import sys, os
sys.path.insert(0, "/opt/trn_rl_repo")
import numpy as np
import ml_dtypes
from contextlib import ExitStack

import concourse.bass as bass
import concourse.bacc as bacc
import concourse.mybir as mybir
import concourse.tile as tile
from concourse.bass_utils import run_bass_kernel_spmd

BF16 = ml_dtypes.bfloat16

N_NODES = 50000
N_EDGES = 500000
D = 128
H = 256
NCORES = 8
E_SH = N_EDGES // NCORES            # 62500
UNIT = 512                           # edges per inner unit (PSUM free dim)
E_PAD = ((E_SH + UNIT - 1) // UNIT) * UNIT   # 62976
CHUNK = 2048                         # edges per DMA chunk
N_SH = N_NODES // NCORES             # 6250
N_PAD = ((N_SH + UNIT - 1) // UNIT) * UNIT   # 6656

_cache = {}


def _build_edge_program():
    nc = bacc.Bacc(None)
    rowT = nc.dram_tensor("rowT", [128, E_PAD], mybir.dt.bfloat16, kind="ExternalInput")
    colT = nc.dram_tensor("colT", [128, E_PAD], mybir.dt.bfloat16, kind="ExternalInput")
    attrT = nc.dram_tensor("attrT", [128, E_PAD], mybir.dt.bfloat16, kind="ExternalInput")
    w1 = nc.dram_tensor("w1", [128, 3 * H], mybir.dt.bfloat16, kind="ExternalInput")
    w2 = nc.dram_tensor("w2", [128, 2 * D], mybir.dt.bfloat16, kind="ExternalInput")
    bia = nc.dram_tensor("bia", [128, 3], mybir.dt.float32, kind="ExternalInput")
    enT = nc.dram_tensor("enT", [128, E_PAD], mybir.dt.bfloat16, kind="ExternalOutput")

    n_chunks = E_PAD // CHUNK if E_PAD % CHUNK == 0 else None
    # E_PAD=62976 is not a multiple of 2560; use ragged chunk list of UNIT multiples
    chunks = []
    off = 0
    while off < E_PAD:
        sz = min(CHUNK, E_PAD - off)
        chunks.append((off, sz))
        off += sz

    with ExitStack() as ctx:
        tc = ctx.enter_context(tile.TileContext(nc))
        wpool = ctx.enter_context(tc.tile_pool(name="w", bufs=1))
        io = ctx.enter_context(tc.tile_pool(name="io", bufs=2))
        mid = ctx.enter_context(tc.tile_pool(name="mid", bufs=3))
        ps1 = ctx.enter_context(tc.tile_pool(name="ps1", bufs=4, space="PSUM"))
        ps2 = ctx.enter_context(tc.tile_pool(name="ps2", bufs=4, space="PSUM"))

        t_w1 = wpool.tile([128, 3 * H], mybir.dt.bfloat16)
        nc.sync.dma_start(out=t_w1[:], in_=w1[:])
        t_w2 = wpool.tile([128, 2 * D], mybir.dt.bfloat16)
        nc.sync.dma_start(out=t_w2[:], in_=w2[:])
        t_b = wpool.tile([128, 3], mybir.dt.float32)
        nc.sync.dma_start(out=t_b[:], in_=bia[:])

        for (coff, csz) in chunks:
            t_row = io.tile([128, CHUNK], mybir.dt.bfloat16, tag="row")
            t_col = io.tile([128, CHUNK], mybir.dt.bfloat16, tag="col")
            t_attr = io.tile([128, CHUNK], mybir.dt.bfloat16, tag="attr")
            nc.sync.dma_start(out=t_row[:, :csz], in_=rowT[:, coff:coff + csz])
            nc.sync.dma_start(out=t_col[:, :csz], in_=colT[:, coff:coff + csz])
            nc.sync.dma_start(out=t_attr[:, :csz], in_=attrT[:, coff:coff + csz])
            t_eno = mid.tile([128, CHUNK], mybir.dt.bfloat16, tag="eno")
            for u in range(csz // UNIT):
                s = slice(u * UNIT, (u + 1) * UNIT)
                # --- mm1: e_h[Mh] = relu(sum_k W1[k,Mh]^T @ in_k + b1[Mh]) ---
                t_eh = mid.tile([128, 2 * UNIT], mybir.dt.bfloat16, tag="eh")
                for mh in range(2):
                    p1 = ps1.tile([128, UNIT], mybir.dt.float32, tag="p1")
                    for k, t_in in enumerate((t_row, t_col, t_attr)):
                        nc.tensor.matmul(
                            out=p1[:],
                            lhsT=t_w1[:, k * H + mh * 128: k * H + mh * 128 + 128],
                            rhs=t_in[:, s],
                            start=(k == 0),
                            stop=(k == 2),
                        )
                    nc.scalar.activation(
                        out=t_eh[:, mh * UNIT:(mh + 1) * UNIT],
                        in_=p1[:],
                        func=mybir.ActivationFunctionType.Relu,
                        bias=t_b[:, mh:mh + 1],
                    )
                # --- mm2: e_new^T = W2^T @ e_h + b2 ---
                p2 = ps2.tile([128, UNIT], mybir.dt.float32, tag="p2")
                for k in range(2):
                    nc.tensor.matmul(
                        out=p2[:],
                        lhsT=t_w2[:, k * D:(k + 1) * D],
                        rhs=t_eh[:, k * UNIT:(k + 1) * UNIT],
                        start=(k == 0),
                        stop=(k == 1),
                    )
                nc.vector.tensor_scalar_add(out=t_eno[:, u * UNIT:(u + 1) * UNIT], in0=p2[:], scalar1=t_b[:, 2:3])
            nc.sync.dma_start(out=enT[:, coff:coff + csz], in_=t_eno[:, :csz])
    nc.compile()
    return nc


def _build_node_program():
    nc = bacc.Bacc(None)
    xT = nc.dram_tensor("xT", [128, N_PAD], mybir.dt.bfloat16, kind="ExternalInput")
    aggT = nc.dram_tensor("aggT", [128, N_PAD], mybir.dt.bfloat16, kind="ExternalInput")
    w1 = nc.dram_tensor("w1", [128, 2 * H], mybir.dt.bfloat16, kind="ExternalInput")
    w2 = nc.dram_tensor("w2", [128, 2 * D], mybir.dt.bfloat16, kind="ExternalInput")
    bia = nc.dram_tensor("bia", [128, 3], mybir.dt.float32, kind="ExternalInput")
    xnT = nc.dram_tensor("xnT", [128, N_PAD], mybir.dt.float32, kind="ExternalOutput")

    with ExitStack() as ctx:
        tc = ctx.enter_context(tile.TileContext(nc))
        wpool = ctx.enter_context(tc.tile_pool(name="w", bufs=1))
        io = ctx.enter_context(tc.tile_pool(name="io", bufs=2))
        mid = ctx.enter_context(tc.tile_pool(name="mid", bufs=3))
        ps1 = ctx.enter_context(tc.tile_pool(name="ps1", bufs=2, space="PSUM"))
        ps2 = ctx.enter_context(tc.tile_pool(name="ps2", bufs=2, space="PSUM"))

        t_w1 = wpool.tile([128, 2 * H], mybir.dt.bfloat16)
        nc.sync.dma_start(out=t_w1[:], in_=w1[:])
        t_w2 = wpool.tile([128, 2 * D], mybir.dt.bfloat16)
        nc.sync.dma_start(out=t_w2[:], in_=w2[:])
        t_b = wpool.tile([128, 3], mybir.dt.float32)
        nc.sync.dma_start(out=t_b[:], in_=bia[:])

        off = 0
        while off < N_PAD:
            csz = min(CHUNK, N_PAD - off)
            t_x = io.tile([128, CHUNK], mybir.dt.bfloat16, tag="x")
            t_a = io.tile([128, CHUNK], mybir.dt.bfloat16, tag="a")
            nc.sync.dma_start(out=t_x[:, :csz], in_=xT[:, off:off + csz])
            nc.sync.dma_start(out=t_a[:, :csz], in_=aggT[:, off:off + csz])
            for u in range(csz // UNIT):
                s = slice(u * UNIT, (u + 1) * UNIT)
                t_nh = mid.tile([128, 2 * UNIT], mybir.dt.bfloat16, tag="nh")
                for mh in range(2):
                    p1 = ps1.tile([128, UNIT], mybir.dt.float32, tag="p1")
                    for k, t_in in enumerate((t_x, t_a)):
                        nc.tensor.matmul(
                            out=p1[:],
                            lhsT=t_w1[:, k * H + mh * 128: k * H + mh * 128 + 128],
                            rhs=t_in[:, s],
                            start=(k == 0),
                            stop=(k == 1),
                        )
                    nc.scalar.activation(
                        out=t_nh[:, mh * UNIT:(mh + 1) * UNIT],
                        in_=p1[:],
                        func=mybir.ActivationFunctionType.Relu,
                        bias=t_b[:, mh:mh + 1],
                    )
                p2 = ps2.tile([128, UNIT], mybir.dt.float32, tag="p2")
                for k in range(2):
                    nc.tensor.matmul(
                        out=p2[:],
                        lhsT=t_w2[:, k * D:(k + 1) * D],
                        rhs=t_nh[:, k * UNIT:(k + 1) * UNIT],
                        start=(k == 0),
                        stop=(k == 1),
                    )
                t_xn = mid.tile([128, UNIT], mybir.dt.float32, tag="xn")
                nc.vector.tensor_scalar_add(out=t_xn[:], in0=p2[:], scalar1=t_b[:, 2:3])
                nc.sync.dma_start(out=xnT[:, off + u * UNIT: off + (u + 1) * UNIT], in_=t_xn[:])
            off += csz
    nc.compile()
    return nc


def _get_programs():
    if "edge" not in _cache:
        _cache["edge"] = _build_edge_program()
        _cache["node"] = _build_node_program()
    return _cache["edge"], _cache["node"]


def _pad_cols(a: np.ndarray, width: int) -> np.ndarray:
    out = np.zeros((a.shape[0], width), dtype=a.dtype)
    out[:, : a.shape[1]] = a
    return out


def kernel(x, edge_index, edge_attr, eW1, eb1, eW2, eb2, nW1, nb1, nW2, nb2,
           trace=False, _timing=None):
    x = np.asarray(x); edge_index = np.asarray(edge_index); edge_attr = np.asarray(edge_attr)
    eW1 = np.asarray(eW1); eb1 = np.asarray(eb1); eW2 = np.asarray(eW2); eb2 = np.asarray(eb2)
    nW1 = np.asarray(nW1); nb1 = np.asarray(nb1); nW2 = np.asarray(nW2); nb2 = np.asarray(nb2)
    row, col = edge_index[0].astype(np.int64), edge_index[1].astype(np.int64)

    nc_edge, nc_node = _get_programs()

    # ---- host prep (sharding): feature-major bf16 operands ----
    xT_bf = np.ascontiguousarray(x.astype(BF16).T)               # [128, N]
    attr_bf = edge_attr.astype(BF16)                              # [E, 128]

    # edge-program weights: w1 [128, 3*H] = eW1 K-chunks; w2 [128, 2*D]
    w1_e = np.ascontiguousarray(eW1.astype(BF16).reshape(3, 128, H).transpose(1, 0, 2)).reshape(128, 3 * H)
    w2_e = np.ascontiguousarray(eW2.astype(BF16).reshape(2, 128, D).transpose(1, 0, 2)).reshape(128, 2 * D)
    bia_e = np.stack([eb1[:128], eb1[128:], eb2], axis=1).astype(np.float32)  # [128, 3]

    in_maps = []
    for c in range(NCORES):
        e0, e1 = c * E_SH, (c + 1) * E_SH
        rc, cc = row[e0:e1], col[e0:e1]
        in_maps.append(dict(
            rowT=_pad_cols(xT_bf[:, rc], E_PAD),
            colT=_pad_cols(xT_bf[:, cc], E_PAD),
            attrT=_pad_cols(np.ascontiguousarray(attr_bf[e0:e1].T), E_PAD),
            w1=w1_e, w2=w2_e, bia=bia_e,
        ))

    res1 = run_bass_kernel_spmd(nc_edge, in_maps, core_ids=list(range(NCORES)),
                                trace=trace)
    if _timing is not None:
        _timing.append(res1)

    # ---- assemble e_new + host segment-sum ----
    e_new = np.empty((N_EDGES, D), dtype=np.float32)
    for c in range(NCORES):
        e_new[c * E_SH:(c + 1) * E_SH] = res1.results[c]["enT"][:, :E_SH].T.astype(np.float32)

    perm = np.argsort(col, kind="stable")
    sc = col[perm]
    starts = np.flatnonzero(np.r_[True, sc[1:] != sc[:-1]])
    sums = np.add.reduceat(e_new[perm], starts, axis=0)
    agg = np.zeros((N_NODES, D), dtype=np.float32)
    agg[sc[starts]] = sums

    # ---- node program ----
    aggT_bf = np.ascontiguousarray(agg.astype(BF16).T)            # [128, N]
    w1_n = np.ascontiguousarray(nW1.astype(BF16).reshape(2, 128, H).transpose(1, 0, 2)).reshape(128, 2 * H)
    w2_n = np.ascontiguousarray(nW2.astype(BF16).reshape(2, 128, D).transpose(1, 0, 2)).reshape(128, 2 * D)
    bia_n = np.stack([nb1[:128], nb1[128:], nb2], axis=1).astype(np.float32)

    in_maps2 = []
    for c in range(NCORES):
        n0, n1 = c * N_SH, (c + 1) * N_SH
        in_maps2.append(dict(
            xT=_pad_cols(xT_bf[:, n0:n1], N_PAD),
            aggT=_pad_cols(aggT_bf[:, n0:n1], N_PAD),
            w1=w1_n, w2=w2_n, bia=bia_n,
        ))
    res2 = run_bass_kernel_spmd(nc_node, in_maps2, core_ids=list(range(NCORES)),
                                trace=trace)
    if _timing is not None:
        _timing.append(res2)

    x_new = np.empty((N_NODES, D), dtype=np.float32)
    for c in range(NCORES):
        x_new[c * N_SH:(c + 1) * N_SH] = res2.results[c]["xnT"][:, :N_SH].T

    return (x_new, e_new)


# revision 6
# speedup vs baseline: 1.0195x; 1.0014x over previous
"""Trainium2 Bass kernel for a GNN MetaLayer (edge MLP -> segment-sum -> node MLP).

Distribution: edge-parallel across 8 NeuronCores (contiguous 62500-edge shards,
x + weights replicated), node MLP node-parallel (contiguous 6250-node shards).
Device does all matmul work feature-major in bf16 with fp32 PSUM accumulation;
host prepares feature-major operands and performs the (memory-bound) segment-sum
between the two launches via sorted reduceat.
"""
import sys, os
sys.path.insert(0, "/opt/trn_rl_repo")
import numpy as np
import ml_dtypes
from contextlib import ExitStack

import concourse.bass as bass
import concourse.bacc as bacc
import concourse.mybir as mybir
import concourse.tile as tile
from concourse.bass_utils import run_bass_kernel_spmd

BF16 = ml_dtypes.bfloat16

N_NODES = 50000
N_EDGES = 500000
D = 128
H = 256
NCORES = 8
E_SH = N_EDGES // NCORES            # 62500
UNIT = 512                           # edges per inner unit (PSUM free dim)
E_PAD = ((E_SH + UNIT - 1) // UNIT) * UNIT   # 62976
CHUNK = 2048                         # edges per DMA chunk
N_SH = N_NODES // NCORES             # 6250
N_PAD = ((N_SH + UNIT - 1) // UNIT) * UNIT   # 6656

_cache = {}


def _build_edge_program():
    nc = bacc.Bacc(None)
    rowT = nc.dram_tensor("rowT", [128, E_PAD], mybir.dt.bfloat16, kind="ExternalInput")
    colT = nc.dram_tensor("colT", [128, E_PAD], mybir.dt.bfloat16, kind="ExternalInput")
    attrT = nc.dram_tensor("attrT", [128, E_PAD], mybir.dt.bfloat16, kind="ExternalInput")
    w1 = nc.dram_tensor("w1", [128, 3 * H], mybir.dt.bfloat16, kind="ExternalInput")
    w2 = nc.dram_tensor("w2", [128, 2 * D], mybir.dt.bfloat16, kind="ExternalInput")
    bia = nc.dram_tensor("bia", [128, 3], mybir.dt.float32, kind="ExternalInput")
    enT = nc.dram_tensor("enT", [128, E_PAD], mybir.dt.bfloat16, kind="ExternalOutput")

    n_chunks = E_PAD // CHUNK if E_PAD % CHUNK == 0 else None
    # E_PAD=62976 is not a multiple of 2560; use ragged chunk list of UNIT multiples
    chunks = []
    off = 0
    while off < E_PAD:
        sz = min(CHUNK, E_PAD - off)
        chunks.append((off, sz))
        off += sz

    with ExitStack() as ctx:
        tc = ctx.enter_context(tile.TileContext(nc))
        wpool = ctx.enter_context(tc.tile_pool(name="w", bufs=1))
        io = ctx.enter_context(tc.tile_pool(name="io", bufs=2))
        mid = ctx.enter_context(tc.tile_pool(name="mid", bufs=3))
        ps1 = ctx.enter_context(tc.tile_pool(name="ps1", bufs=4, space="PSUM"))
        ps2 = ctx.enter_context(tc.tile_pool(name="ps2", bufs=4, space="PSUM"))

        t_w1 = wpool.tile([128, 3 * H], mybir.dt.bfloat16)
        nc.sync.dma_start(out=t_w1[:], in_=w1[:])
        t_w2 = wpool.tile([128, 2 * D], mybir.dt.bfloat16)
        nc.sync.dma_start(out=t_w2[:], in_=w2[:])
        t_b = wpool.tile([128, 3], mybir.dt.float32)
        nc.sync.dma_start(out=t_b[:], in_=bia[:])

        for (coff, csz) in chunks:
            t_row = io.tile([128, CHUNK], mybir.dt.bfloat16, tag="row")
            t_col = io.tile([128, CHUNK], mybir.dt.bfloat16, tag="col")
            t_attr = io.tile([128, CHUNK], mybir.dt.bfloat16, tag="attr")
            nc.sync.dma_start(out=t_row[:, :csz], in_=rowT[:, coff:coff + csz])
            nc.sync.dma_start(out=t_col[:, :csz], in_=colT[:, coff:coff + csz])
            nc.sync.dma_start(out=t_attr[:, :csz], in_=attrT[:, coff:coff + csz])
            t_eno = mid.tile([128, CHUNK], mybir.dt.bfloat16, tag="eno")
            for u in range(csz // UNIT):
                s = slice(u * UNIT, (u + 1) * UNIT)
                # --- mm1: e_h[Mh] = relu(sum_k W1[k,Mh]^T @ in_k + b1[Mh]) ---
                t_eh = mid.tile([128, 2 * UNIT], mybir.dt.bfloat16, tag="eh")
                for mh in range(2):
                    p1 = ps1.tile([128, UNIT], mybir.dt.float32, tag="p1")
                    for k, t_in in enumerate((t_row, t_col, t_attr)):
                        nc.tensor.matmul(
                            out=p1[:],
                            lhsT=t_w1[:, k * H + mh * 128: k * H + mh * 128 + 128],
                            rhs=t_in[:, s],
                            start=(k == 0),
                            stop=(k == 2),
                        )
                    nc.scalar.activation(
                        out=t_eh[:, mh * UNIT:(mh + 1) * UNIT],
                        in_=p1[:],
                        func=mybir.ActivationFunctionType.Relu,
                        bias=t_b[:, mh:mh + 1],
                    )
                # --- mm2: e_new^T = W2^T @ e_h + b2 ---
                p2 = ps2.tile([128, UNIT], mybir.dt.float32, tag="p2")
                for k in range(2):
                    nc.tensor.matmul(
                        out=p2[:],
                        lhsT=t_w2[:, k * D:(k + 1) * D],
                        rhs=t_eh[:, k * UNIT:(k + 1) * UNIT],
                        start=(k == 0),
                        stop=(k == 1),
                    )
                nc.vector.tensor_scalar_add(out=t_eno[:, u * UNIT:(u + 1) * UNIT], in0=p2[:], scalar1=t_b[:, 2:3])
            nc.sync.dma_start(out=enT[:, coff:coff + csz], in_=t_eno[:, :csz])
    nc.compile()
    return nc


def _build_node_program():
    nc = bacc.Bacc(None)
    xT = nc.dram_tensor("xT", [128, N_PAD], mybir.dt.bfloat16, kind="ExternalInput")
    aggT = nc.dram_tensor("aggT", [128, N_PAD], mybir.dt.bfloat16, kind="ExternalInput")
    w1 = nc.dram_tensor("w1", [128, 2 * H], mybir.dt.bfloat16, kind="ExternalInput")
    w2 = nc.dram_tensor("w2", [128, 2 * D], mybir.dt.bfloat16, kind="ExternalInput")
    bia = nc.dram_tensor("bia", [128, 3], mybir.dt.float32, kind="ExternalInput")
    xnT = nc.dram_tensor("xnT", [128, N_PAD], mybir.dt.bfloat16, kind="ExternalOutput")

    with ExitStack() as ctx:
        tc = ctx.enter_context(tile.TileContext(nc))
        wpool = ctx.enter_context(tc.tile_pool(name="w", bufs=1))
        io = ctx.enter_context(tc.tile_pool(name="io", bufs=2))
        mid = ctx.enter_context(tc.tile_pool(name="mid", bufs=3))
        ps1 = ctx.enter_context(tc.tile_pool(name="ps1", bufs=2, space="PSUM"))
        ps2 = ctx.enter_context(tc.tile_pool(name="ps2", bufs=2, space="PSUM"))

        t_w1 = wpool.tile([128, 2 * H], mybir.dt.bfloat16)
        nc.sync.dma_start(out=t_w1[:], in_=w1[:])
        t_w2 = wpool.tile([128, 2 * D], mybir.dt.bfloat16)
        nc.sync.dma_start(out=t_w2[:], in_=w2[:])
        t_b = wpool.tile([128, 3], mybir.dt.float32)
        nc.sync.dma_start(out=t_b[:], in_=bia[:])

        off = 0
        while off < N_PAD:
            csz = min(CHUNK, N_PAD - off)
            t_x = io.tile([128, CHUNK], mybir.dt.bfloat16, tag="x")
            t_a = io.tile([128, CHUNK], mybir.dt.bfloat16, tag="a")
            nc.sync.dma_start(out=t_x[:, :csz], in_=xT[:, off:off + csz])
            nc.sync.dma_start(out=t_a[:, :csz], in_=aggT[:, off:off + csz])
            for u in range(csz // UNIT):
                s = slice(u * UNIT, (u + 1) * UNIT)
                t_nh = mid.tile([128, 2 * UNIT], mybir.dt.bfloat16, tag="nh")
                for mh in range(2):
                    p1 = ps1.tile([128, UNIT], mybir.dt.float32, tag="p1")
                    for k, t_in in enumerate((t_x, t_a)):
                        nc.tensor.matmul(
                            out=p1[:],
                            lhsT=t_w1[:, k * H + mh * 128: k * H + mh * 128 + 128],
                            rhs=t_in[:, s],
                            start=(k == 0),
                            stop=(k == 1),
                        )
                    nc.scalar.activation(
                        out=t_nh[:, mh * UNIT:(mh + 1) * UNIT],
                        in_=p1[:],
                        func=mybir.ActivationFunctionType.Relu,
                        bias=t_b[:, mh:mh + 1],
                    )
                p2 = ps2.tile([128, UNIT], mybir.dt.float32, tag="p2")
                for k in range(2):
                    nc.tensor.matmul(
                        out=p2[:],
                        lhsT=t_w2[:, k * D:(k + 1) * D],
                        rhs=t_nh[:, k * UNIT:(k + 1) * UNIT],
                        start=(k == 0),
                        stop=(k == 1),
                    )
                t_xn = mid.tile([128, UNIT], mybir.dt.bfloat16, tag="xn")
                nc.vector.tensor_scalar_add(out=t_xn[:], in0=p2[:], scalar1=t_b[:, 2:3])
                nc.sync.dma_start(out=xnT[:, off + u * UNIT: off + (u + 1) * UNIT], in_=t_xn[:])
            off += csz
    nc.compile()
    return nc


def _get_programs():
    if "edge" not in _cache:
        _cache["edge"] = _build_edge_program()
        _cache["node"] = _build_node_program()
    return _cache["edge"], _cache["node"]


def _pad_cols(a: np.ndarray, width: int) -> np.ndarray:
    out = np.zeros((a.shape[0], width), dtype=a.dtype)
    out[:, : a.shape[1]] = a
    return out


def kernel(x, edge_index, edge_attr, eW1, eb1, eW2, eb2, nW1, nb1, nW2, nb2,
           trace=False, _timing=None):
    x = np.asarray(x); edge_index = np.asarray(edge_index); edge_attr = np.asarray(edge_attr)
    eW1 = np.asarray(eW1); eb1 = np.asarray(eb1); eW2 = np.asarray(eW2); eb2 = np.asarray(eb2)
    nW1 = np.asarray(nW1); nb1 = np.asarray(nb1); nW2 = np.asarray(nW2); nb2 = np.asarray(nb2)
    row, col = edge_index[0].astype(np.int64), edge_index[1].astype(np.int64)

    nc_edge, nc_node = _get_programs()

    # ---- host prep (sharding): feature-major bf16 operands ----
    xT_bf = np.ascontiguousarray(x.astype(BF16).T)               # [128, N]
    attr_bf = edge_attr.astype(BF16)                              # [E, 128]

    # edge-program weights: w1 [128, 3*H] = eW1 K-chunks; w2 [128, 2*D]
    w1_e = np.ascontiguousarray(eW1.astype(BF16).reshape(3, 128, H).transpose(1, 0, 2)).reshape(128, 3 * H)
    w2_e = np.ascontiguousarray(eW2.astype(BF16).reshape(2, 128, D).transpose(1, 0, 2)).reshape(128, 2 * D)
    bia_e = np.stack([eb1[:128], eb1[128:], eb2], axis=1).astype(np.float32)  # [128, 3]

    in_maps = []
    for c in range(NCORES):
        e0, e1 = c * E_SH, (c + 1) * E_SH
        rc, cc = row[e0:e1], col[e0:e1]
        in_maps.append(dict(
            rowT=_pad_cols(xT_bf[:, rc], E_PAD),
            colT=_pad_cols(xT_bf[:, cc], E_PAD),
            attrT=_pad_cols(np.ascontiguousarray(attr_bf[e0:e1].T), E_PAD),
            w1=w1_e, w2=w2_e, bia=bia_e,
        ))

    res1 = run_bass_kernel_spmd(nc_edge, in_maps, core_ids=list(range(NCORES)),
                                trace=trace)
    if _timing is not None:
        _timing.append(res1)

    # ---- assemble e_new + host segment-sum ----
    e_new = np.empty((N_EDGES, D), dtype=np.float32)
    for c in range(NCORES):
        e_new[c * E_SH:(c + 1) * E_SH] = res1.results[c]["enT"][:, :E_SH].T.astype(np.float32)

    perm = np.argsort(col, kind="stable")
    sc = col[perm]
    starts = np.flatnonzero(np.r_[True, sc[1:] != sc[:-1]])
    sums = np.add.reduceat(e_new[perm], starts, axis=0)
    agg = np.zeros((N_NODES, D), dtype=np.float32)
    agg[sc[starts]] = sums

    # ---- node program ----
    aggT_bf = np.ascontiguousarray(agg.astype(BF16).T)            # [128, N]
    w1_n = np.ascontiguousarray(nW1.astype(BF16).reshape(2, 128, H).transpose(1, 0, 2)).reshape(128, 2 * H)
    w2_n = np.ascontiguousarray(nW2.astype(BF16).reshape(2, 128, D).transpose(1, 0, 2)).reshape(128, 2 * D)
    bia_n = np.stack([nb1[:128], nb1[128:], nb2], axis=1).astype(np.float32)

    in_maps2 = []
    for c in range(NCORES):
        n0, n1 = c * N_SH, (c + 1) * N_SH
        in_maps2.append(dict(
            xT=_pad_cols(xT_bf[:, n0:n1], N_PAD),
            aggT=_pad_cols(aggT_bf[:, n0:n1], N_PAD),
            w1=w1_n, w2=w2_n, bia=bia_n,
        ))
    res2 = run_bass_kernel_spmd(nc_node, in_maps2, core_ids=list(range(NCORES)),
                                trace=trace)
    if _timing is not None:
        _timing.append(res2)

    x_new = np.empty((N_NODES, D), dtype=np.float32)
    for c in range(NCORES):
        x_new[c * N_SH:(c + 1) * N_SH] = res2.results[c]["xnT"][:, :N_SH].T.astype(np.float32)

    return (x_new, e_new)
